# revision 32
# baseline (speedup 1.0000x reference)
"""Trainium2 Bass kernel for the ETD1 ODE block (nn_ODEblockW_28922309771809).

Math (identity-split, degree-4 Taylor, step-doubling):
  X    = dt*A = diag(0.05*sigmoid(alpha)) @ (adj - I),   ||X|| ~ 0.073
  Y    = X^2;  m1_L = I + L',  L' = X + Y/2 + Y@(X/6 + Y/24)
  m2   = dt*I + P'',  P'' = dt*(X/2 + Y/6 + Y@(X/24 + Y/120))
  F    = m2@x0 = P''@x0 + dt*x0
  q    = dt*(w*clip(d,0,1))@w.T  (symmetric);  m1_R = e^{dt(wmat-I)}
       = e^{-dt} e^{q} = a*I + R'',  R'' = a*(q + q^2/2 + q^3/6 + q^4/24)
  step:    V = L'@IC + IC ;  IC' = a*V + V@R'' + F
  Phi^2:   M2 = I + L4 (L4 = 2L' + L'^2),  R2 = a^2 I + R4 (R4 = 2a R'' + R''^2)
           F2 = Phi(F) = a*(F + L'@F) + (F + L'@F)@R'' + F
  z = Phi2(Phi2(Phi2(Phi2(Phi(x)))))       (9 steps = 1 single + 4 doubles)

Numerics (numpy emulation vs fp64 reference): 2.1e-3 frob rel err vs the
2e-2 gate (10x margin). All identity terms are applied exactly in fp32
from local column blocks; everything gathered travels in bf16.

Why doubling: 8-rank AllGathers have a ~17-30 us latency floor nearly
independent of size, and each recurrence step must gather the new state.
Doubling halves the number of chain gathers (8 -> 4); the Phi^2 operator
precompute (M2 pass, R''^2 pass, F2) is emitted between the single step's
gather and the first double step, keeping the PE busy (and its HAM clock
warm) while the gather is in flight.
"""

import math
from contextlib import ExitStack

import numpy as np

import concourse.mybir as mybir
import concourse.tile as tile
from concourse import bacc
from concourse.bass_utils import run_bass_kernel_spmd
from concourse.masks import make_identity

F32 = mybir.dt.float32
F32R = mybir.dt.float32r
BF16 = mybir.dt.bfloat16
AL = mybir.AluOpType

N_CORES = 8
P = 128
N = 2048          # nodes
D = 1024          # features
RB = 256          # node rows per core
FB = 256          # wide-tile width
FBR = 128         # feature cols per core
NKC = N // P      # 16
DKC = D // P      # 8
RJ = RB // P      # 2
ABR = math.exp(-0.1)

LGROUP = [list(range(N_CORES))]


def build_nc():
    nc = bacc.Bacc("TRN2", target_bir_lowering=False, debug=False,
                   num_devices=N_CORES)

    am_rows = nc.dram_tensor("am_rows", [RB, N], F32, kind="ExternalInput")
    alpha_blk = nc.dram_tensor("alpha_blk", [RB], F32, kind="ExternalInput")
    x_full = nc.dram_tensor("x_full", [N, D], F32, kind="ExternalInput")
    x0_full = nc.dram_tensor("x0_full", [N, D], F32, kind="ExternalInput")
    x_rows = nc.dram_tensor("x_rows", [RB, D], F32, kind="ExternalInput")
    x0_rows = nc.dram_tensor("x0_rows", [RB, D], F32, kind="ExternalInput")
    wT_full = nc.dram_tensor("wT_full", [D, D], F32, kind="ExternalInput")
    wTc = nc.dram_tensor("wTc", [D, FBR], F32, kind="ExternalInput")
    d_full = nc.dram_tensor("d_full", [D], F32, kind="ExternalInput")
    z_loc = nc.dram_tensor("z_loc", [RB, D], F32, kind="ExternalOutput")

    with tile.TileContext(nc) as tc, ExitStack() as top:
        const = top.enter_context(tc.tile_pool(name="const", bufs=1))
        dram = top.enter_context(tc.tile_pool(name="dram", bufs=1, space="DRAM"))
        psum = top.enter_context(tc.tile_pool(name="psum", bufs=2, space="PSUM"))
        scrp = top.enter_context(tc.tile_pool(name="scrp", bufs=1))
        lout = top.enter_context(tc.tile_pool(name="lout", bufs=1))

        ident = const.tile([P, P], F32)
        make_identity(nc, ident)
        ident_b = const.tile([P, P], BF16)
        nc.vector.tensor_copy(ident_b[:], ident[:])

        def pe_t(dst_slice, src_slice):
            """dst[128,128] = src[128,128].T via PE transpose; the copy-out
            converts dtype if dst differs from src."""
            if src_slice.dtype == F32R:
                src_slice = src_slice.bitcast(F32)
            src_bf = src_slice.dtype == BF16
            ps = psum.tile([P, P], BF16 if src_bf else F32, tag="tr", bufs=2,
                           name="ps_tr")
            nc.tensor.transpose(ps[:], src_slice,
                                ident_b[:] if src_bf else ident[:])
            nc.vector.tensor_copy(dst_slice, ps[:])

        def agather(ccin, name):
            full = dram.tile([N_CORES * ccin.shape[0], ccin.shape[1]],
                             ccin.dtype, addr_space="Shared",
                             name=f"full_{name}")
            nc.gpsimd.collective_compute(
                "AllGather", AL.bypass, replica_groups=LGROUP,
                ins=[ccin.opt()], outs=[full.opt()])
            return full

        # ---- scales ----
        s_sb = const.tile([P, RJ], F32)
        nc.sync.dma_start(s_sb[:], alpha_blk.ap().rearrange("(j p) -> p j", p=P))
        nc.scalar.activation(s_sb[:], s_sb[:], mybir.ActivationFunctionType.Sigmoid)
        nc.vector.tensor_scalar_mul(s_sb[:], s_sb[:], 0.05)

        dc_sb = const.tile([P, DKC], F32)
        nc.sync.dma_start(dc_sb[:], d_full.ap().rearrange("(q p) -> p q", p=P))
        nc.vector.tensor_scalar(dc_sb[:], dc_sb[:], 0.0, 1.0, AL.max, AL.min)

        # =========================================================
        # Phase A: X row block -> X gathers (j-halves, bf16)
        # =========================================================
        pa_st = ExitStack()
        pa = pa_st.enter_context(tc.tile_pool(name="ph_a", bufs=1))
        pr_st = ExitStack()
        pr = pr_st.enter_context(tc.tile_pool(name="ph_r", bufs=1))
        pax_st = ExitStack()
        pax = pax_st.enter_context(tc.tile_pool(name="ph_ax", bufs=1))

        xrow = pax.tile([P, RJ, N], F32)
        ccin_x = dram.tile([RJ * (NKC // 2) * P, FB], BF16, name="ccin_x")
        for j in range(RJ):
            nc.sync.dma_start(xrow[:, j, :], am_rows[j * P:(j + 1) * P, :])
            nc.vector.tensor_scalar_mul(xrow[:, j, :], xrow[:, j, :],
                                        s_sb[:, j:j + 1])
            for t in range(NKC // 2):
                scb = scrp.tile([P, FB], BF16, tag="ccb", bufs=3, name="ccb")
                nc.vector.tensor_copy(scb[:], xrow[:, j, t * FB:(t + 1) * FB])
                nc.scalar.dma_start(
                    ccin_x[(j * (NKC // 2) + t) * P:
                           (j * (NKC // 2) + t + 1) * P, :], scb[:])
        xg = agather(ccin_x, "x")

        # =========================================================
        # R-side: wmat pass -> q; q gather
        # =========================================================
        prw_st = ExitStack()
        prw = prw_st.enter_context(tc.tile_pool(name="ph_rw", bufs=1))

        wt_sb = prw.tile([P, DKC, D], BF16)
        vrb = prw.tile([P, DKC, FBR], BF16)
        for k in range(DKC):
            wrow = prw.tile([P, D], F32, tag="w_in", bufs=2, name="wrow")
            nc.sync.dma_start(wrow[:], wT_full[k * P:(k + 1) * P, :])
            nc.vector.tensor_copy(wt_sb[:, k, :], wrow[:])
            wtc_k = scrp.tile([P, FBR], F32, tag="wtc", bufs=2, name="wtc_k")
            nc.sync.dma_start(wtc_k[:], wTc[k * P:(k + 1) * P, :])
            sc = scrp.tile([P, FBR], F32, tag="wsc", bufs=2, name="wsc")
            nc.vector.tensor_scalar_mul(sc[:], wtc_k[:], dc_sb[:, k:k + 1])
            nc.vector.tensor_copy(vrb[:, k, :], sc[:])

        q_col = pr.tile([P, DKC, FBR], F32)
        q_colb = pr.tile([P, DKC, FBR], BF16)
        for m in range(DKC):
            ps = psum.tile([P, FBR], F32, tag="mmf", bufs=2, name="ps_f")
            for k in range(DKC):
                nc.tensor.matmul(ps[:], wt_sb[:, k, m * P:(m + 1) * P],
                                 vrb[:, k, :], start=(k == 0), stop=(k == DKC - 1))
            nc.vector.tensor_scalar_mul(q_col[:, m, :], ps[:], 0.1)
            nc.vector.tensor_copy(q_colb[:, m, :], q_col[:, m, :])

        def feat_gather(colb, name):
            rowb = pr.tile([P, D], BF16, tag="f_rowb", bufs=2, name=f"rb_{name}")
            for k in range(DKC):
                pe_t(rowb[:, k * P:(k + 1) * P], colb[:, k, :])
            ccin = dram.tile([(DKC // 2) * P, FB], BF16, name=f"ccin_{name}")
            for t in range(DKC // 2):
                nc.scalar.dma_start(ccin[t * P:(t + 1) * P, :],
                                    rowb[:, t * FB:(t + 1) * FB])
            return agather(ccin, name)

        q_g = feat_gather(q_colb, "q")
        prw_st.close()

        # =========================================================
        # Phase B: xt; X^2 pass; Y gathers
        # =========================================================
        xt = pa.tile([P, NKC, FB], F32)
        for k in range(NKC):
            for j in range(RJ):
                pe_t(xt[:, k, j * P:(j + 1) * P], xrow[:, j, k * P:(k + 1) * P])
        pax_st.close()
        slabn_st = ExitStack()
        slabn = slabn_st.enter_context(tc.tile_pool(name="slab_n", bufs=1))
        paxb_st = ExitStack()
        paxb = paxb_st.enter_context(tc.tile_pool(name="ph_axb", bufs=1))
        xt_b = paxb.tile([P, NKC, FB], BF16)
        nc.vector.tensor_copy(xt_b[:], xt[:])

        b1e_b = pa.tile([P, NKC, FB], BF16)
        b1p_b = pa.tile([P, NKC, FB], BF16)

        def nslab_load(g, mp, tag):
            """[128, 8ranks, 2j, 256] slab for m-pair mp from a single
            (j,t)-tiled gather of an n x n matrix."""
            sl = slabn.tile([P, N_CORES, RJ, FB], BF16, tag=tag, bufs=3,
                            name=f"slab_{tag}")
            a = g[:].rearrange("(c j t p) n -> p c j t n", c=N_CORES, j=RJ,
                               t=NKC // 2, p=P)
            nc.sync.dma_start(sl[:], a[:, :, :, mp, :])
            return sl

        def npass(g, rhs_list, evict, tag):
            for mp in range(NKC // 2):
                sl = nslab_load(g, mp, tag)
                for mh in range(2):
                    m = mp * 2 + mh
                    pss = [psum.tile([P, FB], F32, tag=f"mm{i}", bufs=2,
                                     name=f"ps_mm{i}")
                           for i in range(len(rhs_list))]
                    nk = 0
                    for j in range(RJ):
                        for c in range(N_CORES):
                            lt = sl[:, c, j, mh * P:(mh + 1) * P]
                            for ps, rhs in zip(pss, rhs_list):
                                nc.tensor.matmul(ps[:], lt, rhs[:, c * RJ + j, :],
                                                 start=(nk == 0),
                                                 stop=(nk == NKC - 1))
                            nk += 1
                    evict(m, pss)

        def ev_x2(m, pss):
            sc = scrp.tile([P, FB], F32, tag="nco", bufs=3, name="nco")
            nc.vector.tensor_scalar_mul(sc[:], pss[0][:], 1.0 / 24.0)
            nc.vector.scalar_tensor_tensor(sc[:], xt[:, m, :], 1.0 / 6.0,
                                           sc[:], AL.mult, AL.add)
            nc.vector.tensor_copy(b1e_b[:, m, :], sc[:])
            nc.vector.tensor_scalar_mul(sc[:], pss[0][:], 1.0 / 120.0)
            nc.vector.scalar_tensor_tensor(sc[:], xt[:, m, :], 1.0 / 24.0,
                                           sc[:], AL.mult, AL.add)
            nc.vector.tensor_copy(b1p_b[:, m, :], sc[:])
        npass(xg, [xt_b], ev_x2, "xslab")
        paxb_st.close()


        # =========================================================
        # R-side Horner in q: Yq, q^3, q^4 passes (all off q_g); r2 gather
        # =========================================================
        slabf_st = ExitStack()
        slabf = slabf_st.enter_context(tc.tile_pool(name="slab_f", bufs=1))

        def fpass(g, rhs, evict, tag):
            for mp in range(DKC // 2):
                sl = slabf.tile([P, DKC, FB], BF16, tag=tag, bufs=2,
                                name=f"slab_{tag}")
                a = g[:].rearrange("(c t p) n -> p c t n", c=N_CORES, p=P)
                nc.sync.dma_start(sl[:], a[:, :, mp, :])
                for mh in range(2):
                    m = mp * 2 + mh
                    ps = psum.tile([P, FBR], F32, tag="mmf", bufs=2, name="ps_f")
                    for k in range(DKC):
                        nc.tensor.matmul(ps[:], sl[:, k, mh * P:(mh + 1) * P],
                                         rhs[:, k, :], start=(k == 0),
                                         stop=(k == DKC - 1))
                    evict(m, ps)

        yq_col = pr.tile([P, DKC, FBR], F32)
        yq_colb = pr.tile([P, DKC, FBR], BF16)

        def ev_yq(m, ps):
            nc.vector.tensor_copy(yq_col[:, m, :], ps[:])
            nc.vector.tensor_copy(yq_colb[:, m, :], ps[:])
        fpass(q_g, q_colb, ev_yq, "fslab")

        q3_col = pr.tile([P, DKC, FBR], F32)
        q3_colb = pr.tile([P, DKC, FBR], BF16)

        def ev_q3(m, ps):
            nc.vector.tensor_copy(q3_col[:, m, :], ps[:])
            nc.vector.tensor_copy(q3_colb[:, m, :], ps[:])
        fpass(q_g, yq_colb, ev_q3, "fslab")

        r2_colb = pr.tile([P, DKC, FBR], BF16)

        def ev_r2(m, ps):
            sc = scrp.tile([P, FBR], F32, tag="fco", bufs=2, name="fco")
            nc.vector.tensor_scalar_mul(sc[:], ps[:], ABR / 24.0)
            nc.vector.scalar_tensor_tensor(sc[:], q3_col[:, m, :], ABR / 6.0,
                                           sc[:], AL.mult, AL.add)
            nc.vector.scalar_tensor_tensor(sc[:], yq_col[:, m, :], ABR / 2.0,
                                           sc[:], AL.mult, AL.add)
            nc.vector.scalar_tensor_tensor(sc[:], q_col[:, m, :], ABR,
                                           sc[:], AL.mult, AL.add)
            nc.vector.tensor_copy(r2_colb[:, m, :], sc[:])
        fpass(q_g, q3_colb, ev_r2, "fslab")
        r2_g = feat_gather(r2_colb, "r2")
        slabf_st.close()

        # =========================================================
        # E/P pass -> l2t (L'^T col, F32R), p2t (P''^T col, F32R)
        # =========================================================
        # pass A: W = X@B1e, W' = X@B1p ; D1 = X/2 + W, D2 = X/6 + W'
        d1_b = pa.tile([P, NKC, FB], BF16)
        d2_b = pa.tile([P, NKC, FB], BF16)

        def ev_a(m, pss):
            sc = scrp.tile([P, FB], F32, tag="nco", bufs=3, name="nco")
            nc.vector.scalar_tensor_tensor(sc[:], xt[:, m, :], 0.5, pss[0][:],
                                           AL.mult, AL.add)
            nc.vector.tensor_copy(d1_b[:, m, :], sc[:])
            nc.vector.scalar_tensor_tensor(sc[:], xt[:, m, :], 1.0 / 6.0,
                                           pss[1][:], AL.mult, AL.add)
            nc.vector.tensor_copy(d2_b[:, m, :], sc[:])
        npass(xg, [b1e_b, b1p_b], ev_a, "xslab")

        # pass B: L' = X + X@D1 ; P'' = 0.1*(X/2 + X@D2)
        l2t = lout.tile([P, NKC, FB], F32R)
        l2tb = lout.tile([P, NKC, FB], BF16)
        p2t = pa.tile([P, NKC, FB], F32R)
        lrow_b = lout.tile([P, RJ, N], BF16)
        ccin_l = dram.tile([RJ * (NKC // 2) * P, FB], BF16, name="ccin_l")

        def ev_b(m, pss):
            nc.vector.tensor_add(l2t[:, m, :], pss[0][:], xt[:, m, :])
            nc.vector.tensor_copy(l2tb[:, m, :], l2t[:, m, :].bitcast(F32))
            sc = scrp.tile([P, FB], F32, tag="nco", bufs=3, name="nco")
            nc.vector.tensor_scalar_mul(sc[:], pss[1][:], 0.1)
            nc.vector.scalar_tensor_tensor(p2t[:, m, :], xt[:, m, :], 0.05,
                                           sc[:], AL.mult, AL.add)
            for j in range(RJ):
                pe_t(lrow_b[:, j, m * P:(m + 1) * P],
                     l2tb[:, m, j * P:(j + 1) * P])
            if m % 2 == 1:
                t = m // 2
                for j in range(RJ):
                    nc.scalar.dma_start(
                        ccin_l[(j * (NKC // 2) + t) * P:
                               (j * (NKC // 2) + t + 1) * P, :],
                        lrow_b[:, j, (m - 1) * P:(m + 1) * P])
        npass(xg, [d1_b, d2_b], ev_b, "xslab")
        slabn_st.close()
        lg = agather(ccin_l, "lg")

        # =========================================================
        # Forcing: ft = P''-contract(x0) (+0.1*x0^T below); F gather
        # =========================================================
        pf_st = ExitStack()
        pf = pf_st.enter_context(tc.tile_pool(name="ph_f", bufs=1))
        x0colT = pf.tile([P, DKC, FB], F32)
        xcolT = pf.tile([P, DKC, FB], F32)
        for srct, dst in ((x0_rows, x0colT), (x_rows, xcolT)):
            for j in range(RJ):
                rsb = pf.tile([P, D], F32, tag="rows_in", bufs=2, name="rows_in")
                nc.sync.dma_start(rsb[:], srct[j * P:(j + 1) * P, :])
                for m in range(DKC):
                    pe_t(dst[:, m, j * P:(j + 1) * P],
                         rsb[:, m * P:(m + 1) * P])

        slabp_st = ExitStack()
        slabp = slabp_st.enter_context(tc.tile_pool(name="slab_p", bufs=1))
        ft = lout.tile([P, DKC, FB], F32)
        frow_b = pf.tile([P, RJ, D], BF16)
        ccin_f = dram.tile([RJ * (DKC // 2) * P, FB], BF16, name="ccin_f")

        def plain_pass(plain, rhs, evict, tag):
            for m in range(DKC):
                sl = slabp.tile([P, NKC, P], F32R, tag=tag, bufs=2,
                                name=f"slab_{tag}")
                nc.sync.dma_start(
                    sl[:], plain[:, m * P:(m + 1) * P].bitcast(F32R).rearrange(
                        "(k p) n -> p k n", p=P))
                ps = psum.tile([P, FB], F32, tag="mm0", bufs=2, name="ps_mm0")
                for k in range(NKC):
                    nc.tensor.matmul(ps[:], sl[:, k, :], rhs[:, k, :],
                                     start=(k == 0), stop=(k == NKC - 1))
                evict(m, ps)

        def ev_ft(m, ps):
            nc.vector.scalar_tensor_tensor(ft[:, m, :], x0colT[:, m, :], 0.1,
                                           ps[:], AL.mult, AL.add)
            for j in range(RJ):
                pe_t(frow_b[:, j, m * P:(m + 1) * P],
                     ft[:, m, j * P:(j + 1) * P])
            if m % 2 == 1:
                mp = m // 2
                for j in range(RJ):
                    nc.scalar.dma_start(
                        ccin_f[(j * (DKC // 2) + mp) * P:
                               (j * (DKC // 2) + mp + 1) * P, :],
                        frow_b[:, j, (m - 1) * P:(m + 1) * P])
        plain_pass(x0_full, p2t, ev_ft, "icslab0")
        fg = agather(ccin_f, "fg")

        # --- step 0 V: from fp32 x directly ---
        v = lout.tile([P, DKC, FB], F32, tag="v", bufs=2, name="v")
        v_b = lout.tile([P, DKC, FB], BF16, tag="vb", bufs=2, name="v_b")

        def ev_v0(m, ps):
            nc.vector.tensor_add(v[:, m, :], ps[:], xcolT[:, m, :])
            nc.vector.tensor_copy(v_b[:, m, :], v[:, m, :])
        plain_pass(x_full, l2t, ev_v0, "icslab0")
        slabp_st.close()
        pf_st.close()
        pr_st.close()
        pa_st.close()

        # =========================================================
        # Recurrence pools + R'' slabs
        # =========================================================
        pe = top.enter_context(tc.tile_pool(name="ph_e", bufs=1))
        slabic = top.enter_context(tc.tile_pool(name="slab_ic", bufs=1))

        r2_sb = pe.tile([P, DKC, DKC // 2, FB], BF16)
        nc.sync.dma_start(
            r2_sb[:], r2_g[:].rearrange("(c t p) n -> p c t n",
                                        c=N_CORES, p=P))

        def r2_lhsT(k, m):
            return r2_sb[:, k, m // 2, (m % 2) * P:(m % 2 + 1) * P]

        def r_contract(dst, lhsT_fn, vv, vv_b, alpha, f_t):
            for m in range(DKC):
                ps = psum.tile([P, FB], F32, tag="mm1", bufs=2, name="ps_r")
                for k in range(DKC):
                    nc.tensor.matmul(ps[:], lhsT_fn(k, m), vv_b[:, k, :],
                                     start=(k == 0), stop=(k == DKC - 1))
                nc.vector.scalar_tensor_tensor(dst[:, m, :], vv[:, m, :],
                                               alpha, ps[:], AL.mult, AL.add)
                nc.vector.tensor_add(dst[:, m, :], dst[:, m, :], f_t[:, m, :])

        def ship(icnt, name):
            icrow_b = pe.tile([P, RJ, D], BF16, tag="icrow", bufs=2,
                              name="icrow_b")
            ccin = dram.tile([RJ * (DKC // 2) * P, FB], BF16, tag="ccin_ic",
                             bufs=2, name=f"ccin_{name}")
            for m in range(DKC):
                for j in range(RJ):
                    pe_t(icrow_b[:, j, m * P:(m + 1) * P],
                         icnt[:, m, j * P:(j + 1) * P])
                if m % 2 == 1:
                    mp = m // 2
                    for j in range(RJ):
                        nc.scalar.dma_start(
                            ccin[(j * (DKC // 2) + mp) * P:
                                 (j * (DKC // 2) + mp + 1) * P, :],
                            icrow_b[:, j, (m - 1) * P:(m + 1) * P])
            return agather(ccin, name)

        def v_pass(g, rhs_b, vv, vv_b, prev):
            """vv = Full(g)^T-contract with rhs_b, + prev (exact fp32 term)."""
            ga = g[:].rearrange("(c j t2 p) n -> p c j t2 n",
                                c=N_CORES, j=RJ, t2=DKC // 2, p=P)
            for mp in range(DKC // 2):
                sl = slabic.tile([P, N_CORES, RJ, FB], BF16, tag="icslab",
                                 bufs=2, name="slab_ic")
                nc.sync.dma_start(sl[:], ga[:, :, :, mp, :])
                for mh in range(2):
                    m = mp * 2 + mh
                    ps = psum.tile([P, FB], F32, tag="mm0", bufs=2,
                                   name="ps_mm0")
                    nk = 0
                    for c in range(N_CORES):
                        for j in range(RJ):
                            nc.tensor.matmul(
                                ps[:], sl[:, c, j, mh * P:(mh + 1) * P],
                                rhs_b[:, c * RJ + j, :],
                                start=(nk == 0), stop=(nk == NKC - 1))
                            nk += 1
                    nc.vector.tensor_add(vv[:, m, :], ps[:], prev[:, m, :])
                    nc.vector.tensor_copy(vv_b[:, m, :], vv[:, m, :])

        # --- step 0 R-contract + ship ---
        icnt = pe.tile([P, DKC, FB], F32, tag="icnt", bufs=2, name="icnt")
        r_contract(icnt, r2_lhsT, v, v_b, ABR, ft)
        s0_g = ship(icnt, "s0")
        icnt_prev = icnt

        # --- Phi^2 precompute (covers the s0 gather) ---
        # R4 = 2a R'' + R''^2  (local pass off r2_sb)
        r4_full = pe.tile([P, DKC, D], BF16)
        for m in range(DKC):
            for c4 in range(DKC // 2):
                ps = psum.tile([P, FB], F32, tag="mm1", bufs=2, name="ps_r")
                for k in range(DKC):
                    nc.tensor.matmul(ps[:], r2_lhsT(k, m),
                                     r2_sb[:, k, c4, :],
                                     start=(k == 0), stop=(k == DKC - 1))
                sc = scrp.tile([P, FB], F32, tag="nco", bufs=3, name="nco")
                nc.vector.tensor_copy(sc[:], r2_sb[:, m, c4, :])
                nc.vector.scalar_tensor_tensor(sc[:], sc[:], 2.0 * ABR,
                                               ps[:], AL.mult, AL.add)
                nc.vector.tensor_copy(r4_full[:, m, c4 * FB:(c4 + 1) * FB],
                                      sc[:])

        def r4_lhsT(k, m):
            return r4_full[:, k, m * P:(m + 1) * P]

        # M2: L4 = 2L' + L'^2 (npass over the single L' gather)
        l4tb = pe.tile([P, NKC, FB], BF16)
        la = lg[:].rearrange("(c j t p) n -> p c j t n",
                             c=N_CORES, j=RJ, t=NKC // 2, p=P)
        for mp in range(NKC // 2):
            sl = slabic.tile([P, N_CORES, RJ, FB], BF16, tag="icslab",
                             bufs=2, name="slab_ic")
            nc.sync.dma_start(sl[:], la[:, :, :, mp, :])
            for mh in range(2):
                m = mp * 2 + mh
                ps = psum.tile([P, FB], F32, tag="mm0", bufs=2, name="ps_mm0")
                nk = 0
                for j in range(RJ):
                    for c in range(N_CORES):
                        nc.tensor.matmul(ps[:], sl[:, c, j, mh * P:(mh + 1) * P],
                                         l2tb[:, c * RJ + j, :],
                                         start=(nk == 0), stop=(nk == NKC - 1))
                        nk += 1
                sc = scrp.tile([P, FB], F32, tag="nco", bufs=3, name="nco")
                nc.vector.scalar_tensor_tensor(
                    sc[:], l2t[:, m, :].bitcast(F32), 2.0, ps[:],
                    AL.mult, AL.add)
                nc.vector.tensor_copy(l4tb[:, m, :], sc[:])

        # F2 = a*(F + L'F) + (F + L'F)@R'' + F : U = L'@F_gathered + F
        u = lout.tile([P, DKC, FB], F32, tag="v", bufs=2, name="v")
        u_b = lout.tile([P, DKC, FB], BF16, tag="vb", bufs=2, name="v_b")
        v_pass(fg, l2tb, u, u_b, ft)
        ft2 = pe.tile([P, DKC, FB], F32)
        r_contract(ft2, r2_lhsT, u, u_b, ABR, ft)

        # --- 4 double steps ---
        g_prev = s0_g
        icnt_prev = icnt
        A2 = ABR * ABR
        for dstep in range(4):
            vv = lout.tile([P, DKC, FB], F32, tag="v", bufs=2, name="v")
            vv_b = lout.tile([P, DKC, FB], BF16, tag="vb", bufs=2, name="v_b")
            v_pass(g_prev, l4tb, vv, vv_b, icnt_prev)
            icnt = pe.tile([P, DKC, FB], F32, tag="icnt", bufs=2, name="icnt")
            r_contract(icnt, r4_lhsT, vv, vv_b, A2, ft2)
            if dstep < 3:
                g_prev = ship(icnt, f"d{dstep}")
                icnt_prev = icnt
            else:
                for j in range(RJ):
                    for m in range(DKC):
                        zt = scrp.tile([P, FB], F32, tag="nco", bufs=3,
                                       name="nco")
                        pe_t(zt[:, :P], icnt[:, m, j * P:(j + 1) * P])
                        nc.scalar.dma_start(
                            z_loc[j * P:(j + 1) * P, m * P:(m + 1) * P],
                            zt[:, :P])

    nc.compile()
    return nc


_NC_CACHE = []


def _get_nc():
    if not _NC_CACHE:
        _NC_CACHE.append(build_nc())
    return _NC_CACHE[0]


def make_in_maps(inputs):
    x = np.ascontiguousarray(np.asarray(inputs["x"], dtype=np.float32))
    x0 = np.ascontiguousarray(np.asarray(inputs["x0"], dtype=np.float32))
    adj = np.asarray(inputs["adj"], dtype=np.float32)
    alpha = np.ascontiguousarray(np.asarray(inputs["alpha_train"],
                                            dtype=np.float32))
    w = np.asarray(inputs["w"], dtype=np.float32)
    d = np.ascontiguousarray(np.asarray(inputs["d"], dtype=np.float32))

    am = adj - np.eye(N, dtype=np.float32)
    wT = np.ascontiguousarray(w.T)

    in_maps = []
    for c in range(N_CORES):
        r0 = c * RB
        f0 = c * FBR
        in_maps.append({
            "am_rows": np.ascontiguousarray(am[r0:r0 + RB, :]),
            "alpha_blk": np.ascontiguousarray(alpha[r0:r0 + RB]),
            "x_full": x,
            "x0_full": x0,
            "x_rows": np.ascontiguousarray(x[r0:r0 + RB, :]),
            "x0_rows": np.ascontiguousarray(x0[r0:r0 + RB, :]),
            "wT_full": wT,
            "wTc": np.ascontiguousarray(wT[:, f0:f0 + FBR]),
            "d_full": d,
        })
    return in_maps


def kernel(**inputs) -> np.ndarray:
    nc = _get_nc()
    in_maps = make_in_maps(inputs)
    res = run_bass_kernel_spmd(nc, in_maps, core_ids=list(range(N_CORES)))
    z = np.concatenate([res.results[c]["z_loc"] for c in range(N_CORES)], axis=0)
    return np.ascontiguousarray(z.astype(np.float32))


if __name__ == "__main__":
    rng = np.random.default_rng(0)
    ins = {
        "x": rng.standard_normal((N, D)).astype(np.float32),
        "x0": rng.standard_normal((N, D)).astype(np.float32),
        "adj": (rng.random((N, N)) / N).astype(np.float32),
        "alpha_train": rng.standard_normal((N,)).astype(np.float32),
        "w": (np.eye(D) + 0.02 * rng.standard_normal((D, D))).astype(np.float32),
        "d": rng.random((D,)).astype(np.float32),
    }
    out = kernel(**ins)
    print("kernel output:", out.shape, out.dtype, float(np.linalg.norm(out)))


# revision 33
# speedup vs baseline: 1.0581x; 1.0581x over previous
"""Trainium2 Bass kernel for the ETD1 ODE block (nn_ODEblockW_28922309771809).

Math (identity-split, degree-4 Taylor, step-doubling):
  X    = dt*A = diag(0.05*sigmoid(alpha)) @ (adj - I),   ||X|| ~ 0.073
  Y    = X^2;  m1_L = I + L',  L' = X + Y/2 + Y@(X/6 + Y/24)
  m2   = dt*I + P'',  P'' = dt*(X/2 + Y/6 + Y@(X/24 + Y/120))
  F    = m2@x0 = P''@x0 + dt*x0
  q    = dt*(w*clip(d,0,1))@w.T  (symmetric);  m1_R = e^{dt(wmat-I)}
       = e^{-dt} e^{q} = a*I + R'',  R'' = a*(q + q^2/2 + q^3/6 + q^4/24)
  step:    V = L'@IC + IC ;  IC' = a*V + V@R'' + F
  Phi^2:   M2 = I + L4 (L4 = 2L' + L'^2),  R2 = a^2 I + R4 (R4 = 2a R'' + R''^2)
           F2 = Phi(F) = a*(F + L'@F) + (F + L'@F)@R'' + F
  z = Phi2(Phi2(Phi2(Phi2(Phi(x)))))       (9 steps = 1 single + 4 doubles)

Numerics (numpy emulation vs fp64 reference): 2.1e-3 frob rel err vs the
2e-2 gate (10x margin). All identity terms are applied exactly in fp32
from local column blocks; everything gathered travels in bf16.

Why doubling: 8-rank AllGathers have a ~17-30 us latency floor nearly
independent of size, and each recurrence step must gather the new state.
Doubling halves the number of chain gathers (8 -> 4); the Phi^2 operator
precompute (M2 pass, R''^2 pass, F2) is emitted between the single step's
gather and the first double step, keeping the PE busy (and its HAM clock
warm) while the gather is in flight.
"""

import math
from contextlib import ExitStack

import numpy as np

import concourse.mybir as mybir
import concourse.tile as tile
from concourse import bacc
from concourse.bass_utils import run_bass_kernel_spmd
from concourse.masks import make_identity

F32 = mybir.dt.float32
F32R = mybir.dt.float32r
BF16 = mybir.dt.bfloat16
AL = mybir.AluOpType

N_CORES = 8
P = 128
N = 2048          # nodes
D = 1024          # features
RB = 256          # node rows per core
FB = 256          # wide-tile width
FBR = 128         # feature cols per core
NKC = N // P      # 16
DKC = D // P      # 8
RJ = RB // P      # 2
ABR = math.exp(-0.1)

LGROUP = [list(range(N_CORES))]


def build_nc():
    nc = bacc.Bacc("TRN2", target_bir_lowering=False, debug=False,
                   num_devices=N_CORES)

    am_rows = nc.dram_tensor("am_rows", [RB, N], F32, kind="ExternalInput")
    alpha_blk = nc.dram_tensor("alpha_blk", [RB], F32, kind="ExternalInput")
    x_full = nc.dram_tensor("x_full", [N, D], F32, kind="ExternalInput")
    x0_full = nc.dram_tensor("x0_full", [N, D], F32, kind="ExternalInput")
    x_rows = nc.dram_tensor("x_rows", [RB, D], F32, kind="ExternalInput")
    x0_rows = nc.dram_tensor("x0_rows", [RB, D], F32, kind="ExternalInput")
    wT_full = nc.dram_tensor("wT_full", [D, D], F32, kind="ExternalInput")
    wTc = nc.dram_tensor("wTc", [D, FBR], F32, kind="ExternalInput")
    d_full = nc.dram_tensor("d_full", [D], F32, kind="ExternalInput")
    z_loc = nc.dram_tensor("z_loc", [RB, D], F32, kind="ExternalOutput")

    with tile.TileContext(nc) as tc, ExitStack() as top:
        const = top.enter_context(tc.tile_pool(name="const", bufs=1))
        dram = top.enter_context(tc.tile_pool(name="dram", bufs=1, space="DRAM"))
        psum = top.enter_context(tc.tile_pool(name="psum", bufs=2, space="PSUM"))
        scrp = top.enter_context(tc.tile_pool(name="scrp", bufs=1))
        lout = top.enter_context(tc.tile_pool(name="lout", bufs=1))

        ident = const.tile([P, P], F32)
        make_identity(nc, ident)
        ident_b = const.tile([P, P], BF16)
        nc.vector.tensor_copy(ident_b[:], ident[:])

        def pe_t(dst_slice, src_slice):
            """dst[128,128] = src[128,128].T via PE transpose; the copy-out
            converts dtype if dst differs from src."""
            if src_slice.dtype == F32R:
                src_slice = src_slice.bitcast(F32)
            src_bf = src_slice.dtype == BF16
            ps = psum.tile([P, P], BF16 if src_bf else F32, tag="tr", bufs=2,
                           name="ps_tr")
            nc.tensor.transpose(ps[:], src_slice,
                                ident_b[:] if src_bf else ident[:])
            nc.vector.tensor_copy(dst_slice, ps[:])

        def agather(ccin, name):
            full = dram.tile([N_CORES * ccin.shape[0], ccin.shape[1]],
                             ccin.dtype, addr_space="Shared",
                             name=f"full_{name}")
            nc.gpsimd.collective_compute(
                "AllGather", AL.bypass, replica_groups=LGROUP,
                ins=[ccin.opt()], outs=[full.opt()])
            return full

        # ---- scales ----
        s_sb = const.tile([P, RJ], F32)
        nc.sync.dma_start(s_sb[:], alpha_blk.ap().rearrange("(j p) -> p j", p=P))
        nc.scalar.activation(s_sb[:], s_sb[:], mybir.ActivationFunctionType.Sigmoid)
        nc.vector.tensor_scalar_mul(s_sb[:], s_sb[:], 0.05)

        dc_sb = const.tile([P, DKC], F32)
        nc.sync.dma_start(dc_sb[:], d_full.ap().rearrange("(q p) -> p q", p=P))
        nc.vector.tensor_scalar(dc_sb[:], dc_sb[:], 0.0, 1.0, AL.max, AL.min)

        # =========================================================
        # Phase A: X row block -> X gathers (j-halves, bf16)
        # =========================================================
        pa_st = ExitStack()
        pa = pa_st.enter_context(tc.tile_pool(name="ph_a", bufs=1))
        pr_st = ExitStack()
        pr = pr_st.enter_context(tc.tile_pool(name="ph_r", bufs=1))
        pax_st = ExitStack()
        pax = pax_st.enter_context(tc.tile_pool(name="ph_ax", bufs=1))

        xrow = pax.tile([P, RJ, N], F32)
        ccin_x = dram.tile([RJ * (NKC // 2) * P, FB], BF16, name="ccin_x")
        for j in range(RJ):
            nc.sync.dma_start(xrow[:, j, :], am_rows[j * P:(j + 1) * P, :])
            nc.vector.tensor_scalar_mul(xrow[:, j, :], xrow[:, j, :],
                                        s_sb[:, j:j + 1])
            for t in range(NKC // 2):
                scb = scrp.tile([P, FB], BF16, tag="ccb", bufs=3, name="ccb")
                nc.vector.tensor_copy(scb[:], xrow[:, j, t * FB:(t + 1) * FB])
                nc.scalar.dma_start(
                    ccin_x[(j * (NKC // 2) + t) * P:
                           (j * (NKC // 2) + t + 1) * P, :], scb[:])
        xg = agather(ccin_x, "x")

        # =========================================================
        # R-side: wmat pass -> q; q gather
        # =========================================================
        prw_st = ExitStack()
        prw = prw_st.enter_context(tc.tile_pool(name="ph_rw", bufs=1))

        wt_sb = prw.tile([P, DKC, D], BF16)
        vrb = prw.tile([P, DKC, FBR], BF16)
        for k in range(DKC):
            wrow = prw.tile([P, D], F32, tag="w_in", bufs=2, name="wrow")
            nc.sync.dma_start(wrow[:], wT_full[k * P:(k + 1) * P, :])
            nc.vector.tensor_copy(wt_sb[:, k, :], wrow[:])
            wtc_k = scrp.tile([P, FBR], F32, tag="wtc", bufs=2, name="wtc_k")
            nc.sync.dma_start(wtc_k[:], wTc[k * P:(k + 1) * P, :])
            sc = scrp.tile([P, FBR], F32, tag="wsc", bufs=2, name="wsc")
            nc.vector.tensor_scalar_mul(sc[:], wtc_k[:], dc_sb[:, k:k + 1])
            nc.vector.tensor_copy(vrb[:, k, :], sc[:])

        q_col = pr.tile([P, DKC, FBR], F32)
        q_colb = pr.tile([P, DKC, FBR], BF16)
        for m in range(DKC):
            ps = psum.tile([P, FBR], F32, tag="mmf", bufs=2, name="ps_f")
            for k in range(DKC):
                nc.tensor.matmul(ps[:], wt_sb[:, k, m * P:(m + 1) * P],
                                 vrb[:, k, :], start=(k == 0), stop=(k == DKC - 1))
            nc.vector.tensor_scalar_mul(q_col[:, m, :], ps[:], 0.1)
            nc.vector.tensor_copy(q_colb[:, m, :], q_col[:, m, :])

        def feat_gather(colb, name):
            rowb = pr.tile([P, D], BF16, tag="f_rowb", bufs=2, name=f"rb_{name}")
            for k in range(DKC):
                pe_t(rowb[:, k * P:(k + 1) * P], colb[:, k, :])
            ccin = dram.tile([(DKC // 2) * P, FB], BF16, name=f"ccin_{name}")
            for t in range(DKC // 2):
                nc.scalar.dma_start(ccin[t * P:(t + 1) * P, :],
                                    rowb[:, t * FB:(t + 1) * FB])
            return agather(ccin, name)

        q_g = feat_gather(q_colb, "q")
        prw_st.close()

        # =========================================================
        # Phase B: xt; X^2 pass; Y gathers
        # =========================================================
        xt = pa.tile([P, NKC, FB], F32)
        for k in range(NKC):
            for j in range(RJ):
                pe_t(xt[:, k, j * P:(j + 1) * P], xrow[:, j, k * P:(k + 1) * P])
        pax_st.close()
        slabn_st = ExitStack()
        slabn = slabn_st.enter_context(tc.tile_pool(name="slab_n", bufs=1))
        paxb_st = ExitStack()
        paxb = paxb_st.enter_context(tc.tile_pool(name="ph_axb", bufs=1))
        xt_b = paxb.tile([P, NKC, FB], BF16)
        nc.vector.tensor_copy(xt_b[:], xt[:])

        b1e_b = pa.tile([P, NKC, FB], BF16)
        b1p_b = pa.tile([P, NKC, FB], BF16)

        def nslab_load(g, mp, tag):
            """[128, 8ranks, 2j, 256] slab for m-pair mp from a single
            (j,t)-tiled gather of an n x n matrix."""
            sl = slabn.tile([P, N_CORES, RJ, FB], BF16, tag=tag, bufs=3,
                            name=f"slab_{tag}")
            a = g[:].rearrange("(c j t p) n -> p c j t n", c=N_CORES, j=RJ,
                               t=NKC // 2, p=P)
            nc.sync.dma_start(sl[:], a[:, :, :, mp, :])
            return sl

        def npass(g, rhs_list, evict, tag):
            for mp in range(NKC // 2):
                sl = nslab_load(g, mp, tag)
                for mh in range(2):
                    m = mp * 2 + mh
                    pss = [psum.tile([P, FB], F32, tag=f"mm{i}", bufs=2,
                                     name=f"ps_mm{i}")
                           for i in range(len(rhs_list))]
                    nk = 0
                    for j in range(RJ):
                        for c in range(N_CORES):
                            lt = sl[:, c, j, mh * P:(mh + 1) * P]
                            for ps, rhs in zip(pss, rhs_list):
                                nc.tensor.matmul(ps[:], lt, rhs[:, c * RJ + j, :],
                                                 start=(nk == 0),
                                                 stop=(nk == NKC - 1))
                            nk += 1
                    evict(m, pss)

        def ev_x2(m, pss):
            sc = scrp.tile([P, FB], F32, tag="nco", bufs=3, name="nco")
            nc.vector.tensor_scalar_mul(sc[:], pss[0][:], 1.0 / 24.0)
            nc.vector.scalar_tensor_tensor(sc[:], xt[:, m, :], 1.0 / 6.0,
                                           sc[:], AL.mult, AL.add)
            nc.vector.tensor_copy(b1e_b[:, m, :], sc[:])
            nc.vector.tensor_scalar_mul(sc[:], pss[0][:], 1.0 / 120.0)
            nc.vector.scalar_tensor_tensor(sc[:], xt[:, m, :], 1.0 / 24.0,
                                           sc[:], AL.mult, AL.add)
            nc.vector.tensor_copy(b1p_b[:, m, :], sc[:])
        npass(xg, [xt_b], ev_x2, "xslab")
        paxb_st.close()


        # =========================================================
        # R-side Horner in q: Yq, q^3, q^4 passes (all off q_g); r2 gather
        # =========================================================
        slabf_st = ExitStack()
        slabf = slabf_st.enter_context(tc.tile_pool(name="slab_f", bufs=1))

        def fpass(g, rhs, evict, tag):
            for mp in range(DKC // 2):
                sl = slabf.tile([P, DKC, FB], BF16, tag=tag, bufs=2,
                                name=f"slab_{tag}")
                a = g[:].rearrange("(c t p) n -> p c t n", c=N_CORES, p=P)
                nc.sync.dma_start(sl[:], a[:, :, mp, :])
                for mh in range(2):
                    m = mp * 2 + mh
                    ps = psum.tile([P, FBR], F32, tag="mmf", bufs=2, name="ps_f")
                    for k in range(DKC):
                        nc.tensor.matmul(ps[:], sl[:, k, mh * P:(mh + 1) * P],
                                         rhs[:, k, :], start=(k == 0),
                                         stop=(k == DKC - 1))
                    evict(m, ps)

        yq_col = pr.tile([P, DKC, FBR], F32)
        yq_colb = pr.tile([P, DKC, FBR], BF16)

        def ev_yq(m, ps):
            nc.vector.tensor_copy(yq_col[:, m, :], ps[:])
            nc.vector.tensor_copy(yq_colb[:, m, :], ps[:])
        fpass(q_g, q_colb, ev_yq, "fslab")

        q3_col = pr.tile([P, DKC, FBR], F32)
        q3_colb = pr.tile([P, DKC, FBR], BF16)

        def ev_q3(m, ps):
            nc.vector.tensor_copy(q3_col[:, m, :], ps[:])
            nc.vector.tensor_copy(q3_colb[:, m, :], ps[:])
        fpass(q_g, yq_colb, ev_q3, "fslab")

        r2_colb = pr.tile([P, DKC, FBR], BF16)

        def ev_r2(m, ps):
            sc = scrp.tile([P, FBR], F32, tag="fco", bufs=2, name="fco")
            nc.vector.tensor_scalar_mul(sc[:], ps[:], ABR / 24.0)
            nc.vector.scalar_tensor_tensor(sc[:], q3_col[:, m, :], ABR / 6.0,
                                           sc[:], AL.mult, AL.add)
            nc.vector.scalar_tensor_tensor(sc[:], yq_col[:, m, :], ABR / 2.0,
                                           sc[:], AL.mult, AL.add)
            nc.vector.scalar_tensor_tensor(sc[:], q_col[:, m, :], ABR,
                                           sc[:], AL.mult, AL.add)
            nc.vector.tensor_copy(r2_colb[:, m, :], sc[:])
        fpass(q_g, q3_colb, ev_r2, "fslab")
        r2_g = feat_gather(r2_colb, "r2")
        slabf_st.close()

        # =========================================================
        # E/P pass -> l2t (L'^T col, F32R), p2t (P''^T col, F32R)
        # =========================================================
        # pass A: W = X@B1e, W' = X@B1p ; D1 = X/2 + W, D2 = X/6 + W'
        d1_b = pa.tile([P, NKC, FB], BF16)
        d2_b = pa.tile([P, NKC, FB], BF16)

        def ev_a(m, pss):
            sc = scrp.tile([P, FB], F32, tag="nco", bufs=3, name="nco")
            nc.vector.scalar_tensor_tensor(sc[:], xt[:, m, :], 0.5, pss[0][:],
                                           AL.mult, AL.add)
            nc.vector.tensor_copy(d1_b[:, m, :], sc[:])
            nc.vector.scalar_tensor_tensor(sc[:], xt[:, m, :], 1.0 / 6.0,
                                           pss[1][:], AL.mult, AL.add)
            nc.vector.tensor_copy(d2_b[:, m, :], sc[:])
        npass(xg, [b1e_b, b1p_b], ev_a, "xslab")

        # pass B: L' = X + X@D1 ; P'' = 0.1*(X/2 + X@D2)
        l2t = lout.tile([P, NKC, FB], F32R)
        l2tb = lout.tile([P, NKC, FB], BF16)
        p2t = pa.tile([P, NKC, FB], F32R)
        lrow_b = lout.tile([P, RJ, N], BF16)
        ccin_l = dram.tile([RJ * (NKC // 2) * P, FB], BF16, name="ccin_l")

        def ev_b(m, pss):
            nc.vector.tensor_add(l2t[:, m, :], pss[0][:], xt[:, m, :])
            nc.vector.tensor_copy(l2tb[:, m, :], l2t[:, m, :].bitcast(F32))
            sc = scrp.tile([P, FB], F32, tag="nco", bufs=3, name="nco")
            nc.vector.tensor_scalar_mul(sc[:], pss[1][:], 0.1)
            nc.vector.scalar_tensor_tensor(p2t[:, m, :], xt[:, m, :], 0.05,
                                           sc[:], AL.mult, AL.add)
            for j in range(RJ):
                pe_t(lrow_b[:, j, m * P:(m + 1) * P],
                     l2tb[:, m, j * P:(j + 1) * P])
            if m % 2 == 1:
                t = m // 2
                for j in range(RJ):
                    nc.scalar.dma_start(
                        ccin_l[(j * (NKC // 2) + t) * P:
                               (j * (NKC // 2) + t + 1) * P, :],
                        lrow_b[:, j, (m - 1) * P:(m + 1) * P])
        npass(xg, [d1_b, d2_b], ev_b, "xslab")
        slabn_st.close()
        lg = agather(ccin_l, "lg")

        # =========================================================
        # Forcing: ft = P''-contract(x0) (+0.1*x0^T below); F gather
        # =========================================================
        pf_st = ExitStack()
        pf = pf_st.enter_context(tc.tile_pool(name="ph_f", bufs=1))
        x0colT = pf.tile([P, DKC, FB], F32)
        xcolT = pf.tile([P, DKC, FB], F32)
        for srct, dst in ((x0_rows, x0colT), (x_rows, xcolT)):
            for j in range(RJ):
                rsb = pf.tile([P, D], F32, tag="rows_in", bufs=2, name="rows_in")
                nc.sync.dma_start(rsb[:], srct[j * P:(j + 1) * P, :])
                for m in range(DKC):
                    pe_t(dst[:, m, j * P:(j + 1) * P],
                         rsb[:, m * P:(m + 1) * P])

        slabp_st = ExitStack()
        slabp = slabp_st.enter_context(tc.tile_pool(name="slab_p", bufs=1))
        ft = lout.tile([P, DKC, FB], F32)
        frow_b = pf.tile([P, RJ, D], BF16)
        ccin_f = dram.tile([RJ * (DKC // 2) * P, FB], BF16, name="ccin_f")

        def plain_pass(plain, rhs, evict, tag):
            for m in range(DKC):
                sl = slabp.tile([P, NKC, P], F32R, tag=tag, bufs=2,
                                name=f"slab_{tag}")
                nc.sync.dma_start(
                    sl[:], plain[:, m * P:(m + 1) * P].bitcast(F32R).rearrange(
                        "(k p) n -> p k n", p=P))
                ps = psum.tile([P, FB], F32, tag="mm0", bufs=2, name="ps_mm0")
                for k in range(NKC):
                    nc.tensor.matmul(ps[:], sl[:, k, :], rhs[:, k, :],
                                     start=(k == 0), stop=(k == NKC - 1))
                evict(m, ps)

        def ev_ft(m, ps):
            nc.vector.scalar_tensor_tensor(ft[:, m, :], x0colT[:, m, :], 0.1,
                                           ps[:], AL.mult, AL.add)
            for j in range(RJ):
                pe_t(frow_b[:, j, m * P:(m + 1) * P],
                     ft[:, m, j * P:(j + 1) * P])
            if m % 2 == 1:
                mp = m // 2
                for j in range(RJ):
                    nc.scalar.dma_start(
                        ccin_f[(j * (DKC // 2) + mp) * P:
                               (j * (DKC // 2) + mp + 1) * P, :],
                        frow_b[:, j, (m - 1) * P:(m + 1) * P])
        plain_pass(x0_full, p2t, ev_ft, "icslab0")
        fg = agather(ccin_f, "fg")

        # --- step 0 V: from fp32 x directly ---
        v = lout.tile([P, DKC, FB], F32, tag="v", bufs=2, name="v")
        v_b = lout.tile([P, DKC, FB], BF16, tag="vb", bufs=2, name="v_b")

        def ev_v0(m, ps):
            nc.vector.tensor_add(v[:, m, :], ps[:], xcolT[:, m, :])
            nc.vector.tensor_copy(v_b[:, m, :], v[:, m, :])
        plain_pass(x_full, l2t, ev_v0, "icslab0")
        slabp_st.close()
        pf_st.close()
        pr_st.close()
        pa_st.close()

        # =========================================================
        # Recurrence pools + R'' slabs
        # =========================================================
        pe = top.enter_context(tc.tile_pool(name="ph_e", bufs=1))
        slabic = top.enter_context(tc.tile_pool(name="slab_ic", bufs=1))

        r2_sb = pe.tile([P, DKC, DKC // 2, FB], BF16)
        nc.sync.dma_start(
            r2_sb[:], r2_g[:].rearrange("(c t p) n -> p c t n",
                                        c=N_CORES, p=P))

        def r2_lhsT(k, m):
            return r2_sb[:, k, m // 2, (m % 2) * P:(m % 2 + 1) * P]

        def r_contract(dst, lhsT_fn, vv, vv_b, alpha, f_t):
            for m in range(DKC):
                ps = psum.tile([P, FB], F32, tag="mm1", bufs=2, name="ps_r")
                for k in range(DKC):
                    nc.tensor.matmul(ps[:], lhsT_fn(k, m), vv_b[:, k, :],
                                     start=(k == 0), stop=(k == DKC - 1))
                nc.vector.scalar_tensor_tensor(dst[:, m, :], vv[:, m, :],
                                               alpha, ps[:], AL.mult, AL.add)
                nc.vector.tensor_add(dst[:, m, :], dst[:, m, :], f_t[:, m, :])

        def ship(icnt, name):
            icrow_b = pe.tile([P, RJ, D], BF16, tag="icrow", bufs=2,
                              name="icrow_b")
            ccin = dram.tile([RJ * (DKC // 2) * P, FB], BF16, tag="ccin_ic",
                             bufs=2, name=f"ccin_{name}")
            for m in range(DKC):
                for j in range(RJ):
                    pe_t(icrow_b[:, j, m * P:(m + 1) * P],
                         icnt[:, m, j * P:(j + 1) * P])
                if m % 2 == 1:
                    mp = m // 2
                    for j in range(RJ):
                        nc.scalar.dma_start(
                            ccin[(j * (DKC // 2) + mp) * P:
                                 (j * (DKC // 2) + mp + 1) * P, :],
                            icrow_b[:, j, (m - 1) * P:(m + 1) * P])
            return agather(ccin, name)

        def v_pass(g, rhs_b, vv, vv_b, prev):
            """vv = Full(g)^T-contract with rhs_b, + prev (exact fp32 term)."""
            ga = g[:].rearrange("(c j t2 p) n -> p c j t2 n",
                                c=N_CORES, j=RJ, t2=DKC // 2, p=P)
            for mp in range(DKC // 2):
                sl = slabic.tile([P, N_CORES, RJ, FB], BF16, tag="icslab",
                                 bufs=3, name="slab_ic")
                nc.sync.dma_start(sl[:], ga[:, :, :, mp, :])
                for mh in range(2):
                    m = mp * 2 + mh
                    ps = psum.tile([P, FB], F32, tag="mm0", bufs=2,
                                   name="ps_mm0")
                    nk = 0
                    for c in range(N_CORES):
                        for j in range(RJ):
                            nc.tensor.matmul(
                                ps[:], sl[:, c, j, mh * P:(mh + 1) * P],
                                rhs_b[:, c * RJ + j, :],
                                start=(nk == 0), stop=(nk == NKC - 1))
                            nk += 1
                    nc.vector.tensor_add(vv[:, m, :], ps[:], prev[:, m, :])
                    nc.vector.tensor_copy(vv_b[:, m, :], vv[:, m, :])

        # --- step 0 R-contract + ship ---
        icnt = pe.tile([P, DKC, FB], F32, tag="icnt", bufs=2, name="icnt")
        r_contract(icnt, r2_lhsT, v, v_b, ABR, ft)
        s0_g = ship(icnt, "s0")
        icnt_prev = icnt

        # --- Phi^2 precompute (covers the s0 gather) ---
        # R4 = 2a R'' + R''^2  (local pass off r2_sb)
        r4_full = pe.tile([P, DKC, D], BF16)
        for m in range(DKC):
            for c4 in range(DKC // 2):
                ps = psum.tile([P, FB], F32, tag="mm1", bufs=2, name="ps_r")
                for k in range(DKC):
                    nc.tensor.matmul(ps[:], r2_lhsT(k, m),
                                     r2_sb[:, k, c4, :],
                                     start=(k == 0), stop=(k == DKC - 1))
                sc = scrp.tile([P, FB], F32, tag="nco", bufs=3, name="nco")
                nc.vector.tensor_copy(sc[:], r2_sb[:, m, c4, :])
                nc.vector.scalar_tensor_tensor(sc[:], sc[:], 2.0 * ABR,
                                               ps[:], AL.mult, AL.add)
                nc.vector.tensor_copy(r4_full[:, m, c4 * FB:(c4 + 1) * FB],
                                      sc[:])

        def r4_lhsT(k, m):
            return r4_full[:, k, m * P:(m + 1) * P]

        # M2: L4 = 2L' + L'^2 (npass over the single L' gather)
        l4tb = pe.tile([P, NKC, FB], BF16)
        la = lg[:].rearrange("(c j t p) n -> p c j t n",
                             c=N_CORES, j=RJ, t=NKC // 2, p=P)
        for mp in range(NKC // 2):
            sl = slabic.tile([P, N_CORES, RJ, FB], BF16, tag="icslab",
                             bufs=3, name="slab_ic")
            nc.sync.dma_start(sl[:], la[:, :, :, mp, :])
            for mh in range(2):
                m = mp * 2 + mh
                ps = psum.tile([P, FB], F32, tag="mm0", bufs=2, name="ps_mm0")
                nk = 0
                for j in range(RJ):
                    for c in range(N_CORES):
                        nc.tensor.matmul(ps[:], sl[:, c, j, mh * P:(mh + 1) * P],
                                         l2tb[:, c * RJ + j, :],
                                         start=(nk == 0), stop=(nk == NKC - 1))
                        nk += 1
                sc = scrp.tile([P, FB], F32, tag="nco", bufs=3, name="nco")
                nc.vector.scalar_tensor_tensor(
                    sc[:], l2t[:, m, :].bitcast(F32), 2.0, ps[:],
                    AL.mult, AL.add)
                nc.vector.tensor_copy(l4tb[:, m, :], sc[:])

        # F2 = a*(F + L'F) + (F + L'F)@R'' + F : U = L'@F_gathered + F
        u = lout.tile([P, DKC, FB], F32, tag="v", bufs=2, name="v")
        u_b = lout.tile([P, DKC, FB], BF16, tag="vb", bufs=2, name="v_b")
        v_pass(fg, l2tb, u, u_b, ft)
        ft2 = pe.tile([P, DKC, FB], F32)
        r_contract(ft2, r2_lhsT, u, u_b, ABR, ft)

        # --- 4 double steps ---
        g_prev = s0_g
        icnt_prev = icnt
        A2 = ABR * ABR
        for dstep in range(4):
            vv = lout.tile([P, DKC, FB], F32, tag="v", bufs=2, name="v")
            vv_b = lout.tile([P, DKC, FB], BF16, tag="vb", bufs=2, name="v_b")
            v_pass(g_prev, l4tb, vv, vv_b, icnt_prev)
            icnt = pe.tile([P, DKC, FB], F32, tag="icnt", bufs=2, name="icnt")
            r_contract(icnt, r4_lhsT, vv, vv_b, A2, ft2)
            if dstep < 3:
                g_prev = ship(icnt, f"d{dstep}")
                icnt_prev = icnt
            else:
                for j in range(RJ):
                    for m in range(DKC):
                        zt = scrp.tile([P, FB], F32, tag="nco", bufs=3,
                                       name="nco")
                        pe_t(zt[:, :P], icnt[:, m, j * P:(j + 1) * P])
                        nc.scalar.dma_start(
                            z_loc[j * P:(j + 1) * P, m * P:(m + 1) * P],
                            zt[:, :P])

    nc.compile()
    return nc


_NC_CACHE = []


def _get_nc():
    if not _NC_CACHE:
        _NC_CACHE.append(build_nc())
    return _NC_CACHE[0]


def make_in_maps(inputs):
    x = np.ascontiguousarray(np.asarray(inputs["x"], dtype=np.float32))
    x0 = np.ascontiguousarray(np.asarray(inputs["x0"], dtype=np.float32))
    adj = np.asarray(inputs["adj"], dtype=np.float32)
    alpha = np.ascontiguousarray(np.asarray(inputs["alpha_train"],
                                            dtype=np.float32))
    w = np.asarray(inputs["w"], dtype=np.float32)
    d = np.ascontiguousarray(np.asarray(inputs["d"], dtype=np.float32))

    am = adj - np.eye(N, dtype=np.float32)
    wT = np.ascontiguousarray(w.T)

    in_maps = []
    for c in range(N_CORES):
        r0 = c * RB
        f0 = c * FBR
        in_maps.append({
            "am_rows": np.ascontiguousarray(am[r0:r0 + RB, :]),
            "alpha_blk": np.ascontiguousarray(alpha[r0:r0 + RB]),
            "x_full": x,
            "x0_full": x0,
            "x_rows": np.ascontiguousarray(x[r0:r0 + RB, :]),
            "x0_rows": np.ascontiguousarray(x0[r0:r0 + RB, :]),
            "wT_full": wT,
            "wTc": np.ascontiguousarray(wT[:, f0:f0 + FBR]),
            "d_full": d,
        })
    return in_maps


def kernel(**inputs) -> np.ndarray:
    nc = _get_nc()
    in_maps = make_in_maps(inputs)
    res = run_bass_kernel_spmd(nc, in_maps, core_ids=list(range(N_CORES)))
    z = np.concatenate([res.results[c]["z_loc"] for c in range(N_CORES)], axis=0)
    return np.ascontiguousarray(z.astype(np.float32))


if __name__ == "__main__":
    rng = np.random.default_rng(0)
    ins = {
        "x": rng.standard_normal((N, D)).astype(np.float32),
        "x0": rng.standard_normal((N, D)).astype(np.float32),
        "adj": (rng.random((N, N)) / N).astype(np.float32),
        "alpha_train": rng.standard_normal((N,)).astype(np.float32),
        "w": (np.eye(D) + 0.02 * rng.standard_normal((D, D))).astype(np.float32),
        "d": rng.random((D,)).astype(np.float32),
    }
    out = kernel(**ins)
    print("kernel output:", out.shape, out.dtype, float(np.linalg.norm(out)))


# revision 35
# speedup vs baseline: 1.0853x; 1.0256x over previous
"""Trainium2 Bass kernel for the ETD1 ODE block (nn_ODEblockW_28922309771809).

Math (identity-split, degree-4 Taylor, step-doubling):
  X    = dt*A = diag(0.05*sigmoid(alpha)) @ (adj - I),   ||X|| ~ 0.073
  Y    = X^2;  m1_L = I + L',  L' = X + Y/2 + Y@(X/6 + Y/24)
  m2   = dt*I + P'',  P'' = dt*(X/2 + Y/6 + Y@(X/24 + Y/120))
  F    = m2@x0 = P''@x0 + dt*x0
  q    = dt*(w*clip(d,0,1))@w.T  (symmetric);  m1_R = e^{dt(wmat-I)}
       = e^{-dt} e^{q} = a*I + R'',  R'' = a*(q + q^2/2 + q^3/6 + q^4/24)
  step:    V = L'@IC + IC ;  IC' = a*V + V@R'' + F
  Phi^2:   M2 = I + L4 (L4 = 2L' + L'^2),  R2 = a^2 I + R4 (R4 = 2a R'' + R''^2)
           F2 = Phi(F) = a*(F + L'@F) + (F + L'@F)@R'' + F
  z = Phi2(Phi2(Phi2(Phi2(Phi(x)))))       (9 steps = 1 single + 4 doubles)

Numerics (numpy emulation vs fp64 reference): 2.1e-3 frob rel err vs the
2e-2 gate (10x margin). All identity terms are applied exactly in fp32
from local column blocks; everything gathered travels in bf16.

Why doubling: 8-rank AllGathers have a ~17-30 us latency floor nearly
independent of size, and each recurrence step must gather the new state.
Doubling halves the number of chain gathers (8 -> 4); the Phi^2 operator
precompute (M2 pass, R''^2 pass, F2) is emitted between the single step's
gather and the first double step, keeping the PE busy (and its HAM clock
warm) while the gather is in flight.
"""

import math
from contextlib import ExitStack

import numpy as np

import concourse.mybir as mybir
import concourse.tile as tile
from concourse import bacc
from concourse.bass_utils import run_bass_kernel_spmd
from concourse.masks import make_identity

F32 = mybir.dt.float32
F32R = mybir.dt.float32r
BF16 = mybir.dt.bfloat16
AL = mybir.AluOpType

N_CORES = 8
P = 128
N = 2048          # nodes
D = 1024          # features
RB = 256          # node rows per core
FB = 256          # wide-tile width
FBR = 128         # feature cols per core
NKC = N // P      # 16
DKC = D // P      # 8
RJ = RB // P      # 2
ABR = math.exp(-0.1)

LGROUP = [list(range(N_CORES))]


def build_nc():
    nc = bacc.Bacc("TRN2", target_bir_lowering=False, debug=False,
                   num_devices=N_CORES)

    am_rows = nc.dram_tensor("am_rows", [RB, N], F32, kind="ExternalInput")
    alpha_blk = nc.dram_tensor("alpha_blk", [RB], F32, kind="ExternalInput")
    x_full = nc.dram_tensor("x_full", [N, D], F32, kind="ExternalInput")
    x0_full = nc.dram_tensor("x0_full", [N, D], F32, kind="ExternalInput")
    x_rows = nc.dram_tensor("x_rows", [RB, D], F32, kind="ExternalInput")
    x0_rows = nc.dram_tensor("x0_rows", [RB, D], F32, kind="ExternalInput")
    wT_full = nc.dram_tensor("wT_full", [D, D], F32, kind="ExternalInput")
    wTc = nc.dram_tensor("wTc", [D, FBR], F32, kind="ExternalInput")
    d_full = nc.dram_tensor("d_full", [D], F32, kind="ExternalInput")
    z_loc = nc.dram_tensor("z_loc", [RB, D], F32, kind="ExternalOutput")

    with tile.TileContext(nc) as tc, ExitStack() as top:
        const = top.enter_context(tc.tile_pool(name="const", bufs=1))
        dram = top.enter_context(tc.tile_pool(name="dram", bufs=1, space="DRAM"))
        psum = top.enter_context(tc.tile_pool(name="psum", bufs=2, space="PSUM"))
        scrp = top.enter_context(tc.tile_pool(name="scrp", bufs=1))
        lout = top.enter_context(tc.tile_pool(name="lout", bufs=1))

        ident = const.tile([P, P], F32)
        make_identity(nc, ident)
        ident_b = const.tile([P, P], BF16)
        nc.vector.tensor_copy(ident_b[:], ident[:])

        def pe_t(dst_slice, src_slice):
            """dst[128,128] = src[128,128].T via PE transpose; the copy-out
            converts dtype if dst differs from src."""
            if src_slice.dtype == F32R:
                src_slice = src_slice.bitcast(F32)
            src_bf = src_slice.dtype == BF16
            ps = psum.tile([P, P], BF16 if src_bf else F32, tag="tr", bufs=2,
                           name="ps_tr")
            nc.tensor.transpose(ps[:], src_slice,
                                ident_b[:] if src_bf else ident[:])
            nc.vector.tensor_copy(dst_slice, ps[:])

        def agather(ccin, name):
            full = dram.tile([N_CORES * ccin.shape[0], ccin.shape[1]],
                             ccin.dtype, addr_space="Shared",
                             name=f"full_{name}")
            nc.gpsimd.collective_compute(
                "AllGather", AL.bypass, replica_groups=LGROUP,
                ins=[ccin.opt()], outs=[full.opt()])
            return full

        # ---- scales ----
        s_sb = const.tile([P, RJ], F32)
        nc.sync.dma_start(s_sb[:], alpha_blk.ap().rearrange("(j p) -> p j", p=P))
        nc.scalar.activation(s_sb[:], s_sb[:], mybir.ActivationFunctionType.Sigmoid)
        nc.vector.tensor_scalar_mul(s_sb[:], s_sb[:], 0.05)

        dc_sb = const.tile([P, DKC], F32)
        nc.sync.dma_start(dc_sb[:], d_full.ap().rearrange("(q p) -> p q", p=P))
        nc.vector.tensor_scalar(dc_sb[:], dc_sb[:], 0.0, 1.0, AL.max, AL.min)

        # =========================================================
        # Phase A: X row block -> X gathers (j-halves, bf16)
        # =========================================================
        pa_st = ExitStack()
        pa = pa_st.enter_context(tc.tile_pool(name="ph_a", bufs=1))
        pr_st = ExitStack()
        pr = pr_st.enter_context(tc.tile_pool(name="ph_r", bufs=1))
        pax_st = ExitStack()
        pax = pax_st.enter_context(tc.tile_pool(name="ph_ax", bufs=1))

        xrow = pax.tile([P, RJ, N], F32)
        ccin_x = dram.tile([RJ * (NKC // 2) * P, FB], BF16, name="ccin_x")
        for j in range(RJ):
            nc.sync.dma_start(xrow[:, j, :], am_rows[j * P:(j + 1) * P, :])
            nc.vector.tensor_scalar_mul(xrow[:, j, :], xrow[:, j, :],
                                        s_sb[:, j:j + 1])
            for t in range(NKC // 2):
                scb = scrp.tile([P, FB], BF16, tag="ccb", bufs=3, name="ccb")
                nc.vector.tensor_copy(scb[:], xrow[:, j, t * FB:(t + 1) * FB])
                nc.scalar.dma_start(
                    ccin_x[(j * (NKC // 2) + t) * P:
                           (j * (NKC // 2) + t + 1) * P, :], scb[:])
        xg = agather(ccin_x, "x")

        # =========================================================
        # R-side: wmat pass -> q; q gather
        # =========================================================
        prw_st = ExitStack()
        prw = prw_st.enter_context(tc.tile_pool(name="ph_rw", bufs=1))

        wt_sb = prw.tile([P, DKC, D], BF16)
        vrb = prw.tile([P, DKC, FBR], BF16)
        for k in range(DKC):
            wrow = prw.tile([P, D], F32, tag="w_in", bufs=2, name="wrow")
            nc.sync.dma_start(wrow[:], wT_full[k * P:(k + 1) * P, :])
            nc.vector.tensor_copy(wt_sb[:, k, :], wrow[:])
            wtc_k = scrp.tile([P, FBR], F32, tag="wtc", bufs=2, name="wtc_k")
            nc.sync.dma_start(wtc_k[:], wTc[k * P:(k + 1) * P, :])
            sc = scrp.tile([P, FBR], F32, tag="wsc", bufs=2, name="wsc")
            nc.vector.tensor_scalar_mul(sc[:], wtc_k[:], dc_sb[:, k:k + 1])
            nc.vector.tensor_copy(vrb[:, k, :], sc[:])

        q_col = pr.tile([P, DKC, FBR], F32)
        q_colb = pr.tile([P, DKC, FBR], BF16)
        for m in range(DKC):
            ps = psum.tile([P, FBR], F32, tag="mmf", bufs=2, name="ps_f")
            for k in range(DKC):
                nc.tensor.matmul(ps[:], wt_sb[:, k, m * P:(m + 1) * P],
                                 vrb[:, k, :], start=(k == 0), stop=(k == DKC - 1))
            nc.vector.tensor_scalar_mul(q_col[:, m, :], ps[:], 0.1)
            nc.vector.tensor_copy(q_colb[:, m, :], q_col[:, m, :])

        def feat_gather(colb, name):
            rowb = pr.tile([P, D], BF16, tag="f_rowb", bufs=2, name=f"rb_{name}")
            for k in range(DKC):
                pe_t(rowb[:, k * P:(k + 1) * P], colb[:, k, :])
            ccin = dram.tile([(DKC // 2) * P, FB], BF16, name=f"ccin_{name}")
            for t in range(DKC // 2):
                nc.scalar.dma_start(ccin[t * P:(t + 1) * P, :],
                                    rowb[:, t * FB:(t + 1) * FB])
            return agather(ccin, name)

        q_g = feat_gather(q_colb, "q")
        prw_st.close()

        # =========================================================
        # Phase B: xt; X^2 pass; Y gathers
        # =========================================================
        xt = pa.tile([P, NKC, FB], F32)
        for k in range(NKC):
            for j in range(RJ):
                pe_t(xt[:, k, j * P:(j + 1) * P], xrow[:, j, k * P:(k + 1) * P])
        pax_st.close()
        slabn_st = ExitStack()
        slabn = slabn_st.enter_context(tc.tile_pool(name="slab_n", bufs=1))
        paxb_st = ExitStack()
        paxb = paxb_st.enter_context(tc.tile_pool(name="ph_axb", bufs=1))
        xt_b = paxb.tile([P, NKC, FB], BF16)
        nc.vector.tensor_copy(xt_b[:], xt[:])

        b1e_b = pa.tile([P, NKC, FB], BF16)
        b1p_b = pa.tile([P, NKC, FB], BF16)
        d3_b = pa.tile([P, NKC, FB], BF16)

        def nslab_load(g, mp, tag):
            """[128, 8ranks, 2j, 256] slab for m-pair mp from a single
            (j,t)-tiled gather of an n x n matrix."""
            sl = slabn.tile([P, N_CORES, RJ, FB], BF16, tag=tag, bufs=3,
                            name=f"slab_{tag}")
            a = g[:].rearrange("(c j t p) n -> p c j t n", c=N_CORES, j=RJ,
                               t=NKC // 2, p=P)
            nc.sync.dma_start(sl[:], a[:, :, :, mp, :])
            return sl

        def npass(g, rhs_list, evict, tag):
            for mp in range(NKC // 2):
                sl = nslab_load(g, mp, tag)
                for mh in range(2):
                    m = mp * 2 + mh
                    pss = [psum.tile([P, FB], F32, tag=f"mm{i}", bufs=2,
                                     name=f"ps_mm{i}")
                           for i in range(len(rhs_list))]
                    nk = 0
                    for j in range(RJ):
                        for c in range(N_CORES):
                            lt = sl[:, c, j, mh * P:(mh + 1) * P]
                            for ps, rhs in zip(pss, rhs_list):
                                nc.tensor.matmul(ps[:], lt, rhs[:, c * RJ + j, :],
                                                 start=(nk == 0),
                                                 stop=(nk == NKC - 1))
                            nk += 1
                    evict(m, pss)

        def ev_x2(m, pss):
            sc = scrp.tile([P, FB], F32, tag="nco", bufs=3, name="nco")
            nc.vector.tensor_scalar_mul(sc[:], pss[0][:], 1.0 / 24.0)
            nc.vector.scalar_tensor_tensor(sc[:], xt[:, m, :], 1.0 / 6.0,
                                           sc[:], AL.mult, AL.add)
            nc.vector.tensor_copy(b1e_b[:, m, :], sc[:])
            nc.vector.tensor_scalar_mul(sc[:], pss[0][:], 1.0 / 120.0)
            nc.vector.scalar_tensor_tensor(sc[:], xt[:, m, :], 1.0 / 24.0,
                                           sc[:], AL.mult, AL.add)
            nc.vector.tensor_copy(b1p_b[:, m, :], sc[:])
        npass(xg, [xt_b], ev_x2, "xslab")
        paxb_st.close()


        # =========================================================
        # R-side Horner in q: Yq, q^3, q^4 passes (all off q_g); r2 gather
        # =========================================================
        slabf_st = ExitStack()
        slabf = slabf_st.enter_context(tc.tile_pool(name="slab_f", bufs=1))

        def fpass(g, rhs, evict, tag):
            for mp in range(DKC // 2):
                sl = slabf.tile([P, DKC, FB], BF16, tag=tag, bufs=2,
                                name=f"slab_{tag}")
                a = g[:].rearrange("(c t p) n -> p c t n", c=N_CORES, p=P)
                nc.sync.dma_start(sl[:], a[:, :, mp, :])
                for mh in range(2):
                    m = mp * 2 + mh
                    ps = psum.tile([P, FBR], F32, tag="mmf", bufs=2, name="ps_f")
                    for k in range(DKC):
                        nc.tensor.matmul(ps[:], sl[:, k, mh * P:(mh + 1) * P],
                                         rhs[:, k, :], start=(k == 0),
                                         stop=(k == DKC - 1))
                    evict(m, ps)

        yq_col = pr.tile([P, DKC, FBR], F32)
        yq_colb = pr.tile([P, DKC, FBR], BF16)

        def ev_yq(m, ps):
            nc.vector.tensor_copy(yq_col[:, m, :], ps[:])
            nc.vector.tensor_copy(yq_colb[:, m, :], ps[:])
        fpass(q_g, q_colb, ev_yq, "fslab")

        q3_col = pr.tile([P, DKC, FBR], F32)
        q3_colb = pr.tile([P, DKC, FBR], BF16)

        def ev_q3(m, ps):
            nc.vector.tensor_copy(q3_col[:, m, :], ps[:])
            nc.vector.tensor_copy(q3_colb[:, m, :], ps[:])
        fpass(q_g, yq_colb, ev_q3, "fslab")

        r2_colb = pr.tile([P, DKC, FBR], BF16)

        def ev_r2(m, ps):
            sc = scrp.tile([P, FBR], F32, tag="fco", bufs=2, name="fco")
            nc.vector.tensor_scalar_mul(sc[:], ps[:], ABR / 24.0)
            nc.vector.scalar_tensor_tensor(sc[:], q3_col[:, m, :], ABR / 6.0,
                                           sc[:], AL.mult, AL.add)
            nc.vector.scalar_tensor_tensor(sc[:], yq_col[:, m, :], ABR / 2.0,
                                           sc[:], AL.mult, AL.add)
            nc.vector.scalar_tensor_tensor(sc[:], q_col[:, m, :], ABR,
                                           sc[:], AL.mult, AL.add)
            nc.vector.tensor_copy(r2_colb[:, m, :], sc[:])
        fpass(q_g, q3_colb, ev_r2, "fslab")
        r2_g = feat_gather(r2_colb, "r2")
        slabf_st.close()

        # =========================================================
        # E/P pass -> l2t (L'^T col, F32R), p2t (P''^T col, F32R)
        # =========================================================
        # pass A: W = X@B1e, W' = X@B1p ; D1 = X/2 + W, D2 = X/6 + W'
        d1_b = pa.tile([P, NKC, FB], BF16)
        d2_b = pa.tile([P, NKC, FB], BF16)

        def ev_a(m, pss):
            sc = scrp.tile([P, FB], F32, tag="nco", bufs=3, name="nco")
            nc.vector.scalar_tensor_tensor(sc[:], xt[:, m, :], 0.5, pss[0][:],
                                           AL.mult, AL.add)
            nc.vector.tensor_copy(d1_b[:, m, :], sc[:])
            nc.vector.scalar_tensor_tensor(sc[:], xt[:, m, :], 1.0 / 6.0,
                                           pss[1][:], AL.mult, AL.add)
            nc.vector.tensor_copy(d2_b[:, m, :], sc[:])
            nc.vector.tensor_scalar_mul(sc[:], pss[1][:], -160.0)
            nc.vector.scalar_tensor_tensor(sc[:], pss[0][:], 48.0, sc[:],
                                           AL.mult, AL.add)
            nc.vector.scalar_tensor_tensor(sc[:], xt[:, m, :], 2.0, sc[:],
                                           AL.mult, AL.add)
            nc.vector.tensor_copy(d3_b[:, m, :], sc[:])
        npass(xg, [b1e_b, b1p_b], ev_a, "xslab")

        # pass B: L' = X + X@D1 ; P'' = 0.1*(X/2 + X@D2)
        l2t = lout.tile([P, NKC, FB], F32R)
        l2tb = lout.tile([P, NKC, FB], BF16)
        l4tb = lout.tile([P, NKC, FB], BF16)
        p2t = pa.tile([P, NKC, FB], F32R)

        def ev_b(m, pss):
            nc.vector.tensor_add(l2t[:, m, :], pss[0][:], xt[:, m, :])
            nc.vector.tensor_copy(l2tb[:, m, :], l2t[:, m, :].bitcast(F32))
            sc = scrp.tile([P, FB], F32, tag="nco", bufs=3, name="nco")
            nc.vector.tensor_scalar_mul(sc[:], pss[1][:], 0.1)
            nc.vector.scalar_tensor_tensor(p2t[:, m, :], xt[:, m, :], 0.05,
                                           sc[:], AL.mult, AL.add)
        npass(xg, [d1_b, d2_b], ev_b, "xslab")

        # pass B2: L4 = 2X + X@D3  (= deg-4 series of e^{2X} - I)
        def ev_b2(m, pss):
            sc = scrp.tile([P, FB], F32, tag="nco", bufs=3, name="nco")
            nc.vector.scalar_tensor_tensor(sc[:], xt[:, m, :], 2.0, pss[0][:],
                                           AL.mult, AL.add)
            nc.vector.tensor_copy(l4tb[:, m, :], sc[:])
        npass(xg, [d3_b], ev_b2, "xslab")
        slabn_st.close()

        # =========================================================
        # Forcing: ft = P''-contract(x0) (+0.1*x0^T below); F gather
        # =========================================================
        pf_st = ExitStack()
        pf = pf_st.enter_context(tc.tile_pool(name="ph_f", bufs=1))
        x0colT = pf.tile([P, DKC, FB], F32)
        xcolT = pf.tile([P, DKC, FB], F32)
        for srct, dst in ((x0_rows, x0colT), (x_rows, xcolT)):
            for j in range(RJ):
                rsb = pf.tile([P, D], F32, tag="rows_in", bufs=2, name="rows_in")
                nc.sync.dma_start(rsb[:], srct[j * P:(j + 1) * P, :])
                for m in range(DKC):
                    pe_t(dst[:, m, j * P:(j + 1) * P],
                         rsb[:, m * P:(m + 1) * P])

        slabp_st = ExitStack()
        slabp = slabp_st.enter_context(tc.tile_pool(name="slab_p", bufs=1))
        ft = lout.tile([P, DKC, FB], F32)
        frow_b = pf.tile([P, RJ, D], BF16)
        ccin_f = dram.tile([RJ * (DKC // 2) * P, FB], BF16, name="ccin_f")

        def plain_pass(plain, rhs, evict, tag):
            for m in range(DKC):
                sl = slabp.tile([P, NKC, P], F32R, tag=tag, bufs=2,
                                name=f"slab_{tag}")
                nc.sync.dma_start(
                    sl[:], plain[:, m * P:(m + 1) * P].bitcast(F32R).rearrange(
                        "(k p) n -> p k n", p=P))
                ps = psum.tile([P, FB], F32, tag="mm0", bufs=2, name="ps_mm0")
                for k in range(NKC):
                    nc.tensor.matmul(ps[:], sl[:, k, :], rhs[:, k, :],
                                     start=(k == 0), stop=(k == NKC - 1))
                evict(m, ps)

        def ev_ft(m, ps):
            nc.vector.scalar_tensor_tensor(ft[:, m, :], x0colT[:, m, :], 0.1,
                                           ps[:], AL.mult, AL.add)
            for j in range(RJ):
                pe_t(frow_b[:, j, m * P:(m + 1) * P],
                     ft[:, m, j * P:(j + 1) * P])
            if m % 2 == 1:
                mp = m // 2
                for j in range(RJ):
                    nc.scalar.dma_start(
                        ccin_f[(j * (DKC // 2) + mp) * P:
                               (j * (DKC // 2) + mp + 1) * P, :],
                        frow_b[:, j, (m - 1) * P:(m + 1) * P])
        plain_pass(x0_full, p2t, ev_ft, "icslab0")
        fg = agather(ccin_f, "fg")

        # --- step 0 V: from fp32 x directly ---
        v = lout.tile([P, DKC, FB], F32, tag="v", bufs=1, name="v")
        v_b = lout.tile([P, DKC, FB], BF16, tag="vb", bufs=1, name="v_b")

        def ev_v0(m, ps):
            nc.vector.tensor_add(v[:, m, :], ps[:], xcolT[:, m, :])
            nc.vector.tensor_copy(v_b[:, m, :], v[:, m, :])
        plain_pass(x_full, l2t, ev_v0, "icslab0")
        slabp_st.close()
        pf_st.close()
        pr_st.close()
        pa_st.close()

        # =========================================================
        # Recurrence pools + R'' slabs
        # =========================================================
        pe = top.enter_context(tc.tile_pool(name="ph_e", bufs=1))
        slabic = top.enter_context(tc.tile_pool(name="slab_ic", bufs=1))

        r2_sb = pe.tile([P, DKC, DKC // 2, FB], BF16)
        nc.sync.dma_start(
            r2_sb[:], r2_g[:].rearrange("(c t p) n -> p c t n",
                                        c=N_CORES, p=P))

        def r2_lhsT(k, m):
            return r2_sb[:, k, m // 2, (m % 2) * P:(m % 2 + 1) * P]

        def r_contract(dst, lhsT_fn, vv, vv_b, alpha, f_t):
            for m in range(DKC):
                ps = psum.tile([P, FB], F32, tag="mm1", bufs=2, name="ps_r")
                for k in range(DKC):
                    nc.tensor.matmul(ps[:], lhsT_fn(k, m), vv_b[:, k, :],
                                     start=(k == 0), stop=(k == DKC - 1))
                nc.vector.scalar_tensor_tensor(dst[:, m, :], vv[:, m, :],
                                               alpha, ps[:], AL.mult, AL.add)
                nc.vector.tensor_add(dst[:, m, :], dst[:, m, :], f_t[:, m, :])

        def ship(icnt, name):
            icrow_b = pe.tile([P, RJ, D], BF16, tag="icrow", bufs=2,
                              name="icrow_b")
            ccin = dram.tile([RJ * (DKC // 2) * P, FB], BF16, tag="ccin_ic",
                             bufs=2, name=f"ccin_{name}")
            for m in range(DKC):
                for j in range(RJ):
                    pe_t(icrow_b[:, j, m * P:(m + 1) * P],
                         icnt[:, m, j * P:(j + 1) * P])
                if m % 2 == 1:
                    mp = m // 2
                    for j in range(RJ):
                        nc.scalar.dma_start(
                            ccin[(j * (DKC // 2) + mp) * P:
                                 (j * (DKC // 2) + mp + 1) * P, :],
                            icrow_b[:, j, (m - 1) * P:(m + 1) * P])
            return agather(ccin, name)

        def v_pass(g, rhs_b, vv, vv_b, prev):
            """vv = Full(g)^T-contract with rhs_b, + prev (exact fp32 term)."""
            ga = g[:].rearrange("(c j t2 p) n -> p c j t2 n",
                                c=N_CORES, j=RJ, t2=DKC // 2, p=P)
            for mp in range(DKC // 2):
                sl = slabic.tile([P, N_CORES, RJ, FB], BF16, tag="icslab",
                                 bufs=3, name="slab_ic")
                nc.sync.dma_start(sl[:], ga[:, :, :, mp, :])
                for mh in range(2):
                    m = mp * 2 + mh
                    ps = psum.tile([P, FB], F32, tag="mm0", bufs=2,
                                   name="ps_mm0")
                    nk = 0
                    for c in range(N_CORES):
                        for j in range(RJ):
                            nc.tensor.matmul(
                                ps[:], sl[:, c, j, mh * P:(mh + 1) * P],
                                rhs_b[:, c * RJ + j, :],
                                start=(nk == 0), stop=(nk == NKC - 1))
                            nk += 1
                    nc.vector.tensor_add(vv[:, m, :], ps[:], prev[:, m, :])
                    nc.vector.tensor_copy(vv_b[:, m, :], vv[:, m, :])

        # --- step 0 R-contract + ship ---
        icnt = pe.tile([P, DKC, FB], F32, tag="icnt", bufs=2, name="icnt")
        r_contract(icnt, r2_lhsT, v, v_b, ABR, ft)
        s0_g = ship(icnt, "s0")
        icnt_prev = icnt

        # --- Phi^2 precompute (covers the s0 gather) ---
        # R4 = 2a R'' + R''^2  (local pass off r2_sb)
        r4_full = pe.tile([P, DKC, D], BF16)
        for m in range(DKC):
            for c4 in range(DKC // 2):
                ps = psum.tile([P, FB], F32, tag="mm1", bufs=2, name="ps_r")
                for k in range(DKC):
                    nc.tensor.matmul(ps[:], r2_lhsT(k, m),
                                     r2_sb[:, k, c4, :],
                                     start=(k == 0), stop=(k == DKC - 1))
                sc = scrp.tile([P, FB], F32, tag="nco", bufs=3, name="nco")
                nc.vector.tensor_copy(sc[:], r2_sb[:, m, c4, :])
                nc.vector.scalar_tensor_tensor(sc[:], sc[:], 2.0 * ABR,
                                               ps[:], AL.mult, AL.add)
                nc.vector.tensor_copy(r4_full[:, m, c4 * FB:(c4 + 1) * FB],
                                      sc[:])

        def r4_lhsT(k, m):
            return r4_full[:, k, m * P:(m + 1) * P]

        # F2 = a*(F + L'F) + (F + L'F)@R'' + F : U = L'@F_gathered + F
        u = lout.tile([P, DKC, FB], F32, tag="v", bufs=1, name="v")
        u_b = lout.tile([P, DKC, FB], BF16, tag="vb", bufs=1, name="v_b")
        v_pass(fg, l2tb, u, u_b, ft)
        ft2 = pe.tile([P, DKC, FB], F32)
        r_contract(ft2, r2_lhsT, u, u_b, ABR, ft)

        # --- 4 double steps ---
        g_prev = s0_g
        icnt_prev = icnt
        A2 = ABR * ABR
        for dstep in range(4):
            vv = lout.tile([P, DKC, FB], F32, tag="v", bufs=1, name="v")
            vv_b = lout.tile([P, DKC, FB], BF16, tag="vb", bufs=1, name="v_b")
            v_pass(g_prev, l4tb, vv, vv_b, icnt_prev)
            icnt = pe.tile([P, DKC, FB], F32, tag="icnt", bufs=2, name="icnt")
            r_contract(icnt, r4_lhsT, vv, vv_b, A2, ft2)
            if dstep < 3:
                g_prev = ship(icnt, f"d{dstep}")
                icnt_prev = icnt
            else:
                for j in range(RJ):
                    for m in range(DKC):
                        zt = scrp.tile([P, FB], F32, tag="nco", bufs=3,
                                       name="nco")
                        pe_t(zt[:, :P], icnt[:, m, j * P:(j + 1) * P])
                        nc.scalar.dma_start(
                            z_loc[j * P:(j + 1) * P, m * P:(m + 1) * P],
                            zt[:, :P])

    nc.compile()
    return nc


_NC_CACHE = []


def _get_nc():
    if not _NC_CACHE:
        _NC_CACHE.append(build_nc())
    return _NC_CACHE[0]


def make_in_maps(inputs):
    x = np.ascontiguousarray(np.asarray(inputs["x"], dtype=np.float32))
    x0 = np.ascontiguousarray(np.asarray(inputs["x0"], dtype=np.float32))
    adj = np.asarray(inputs["adj"], dtype=np.float32)
    alpha = np.ascontiguousarray(np.asarray(inputs["alpha_train"],
                                            dtype=np.float32))
    w = np.asarray(inputs["w"], dtype=np.float32)
    d = np.ascontiguousarray(np.asarray(inputs["d"], dtype=np.float32))

    am = adj - np.eye(N, dtype=np.float32)
    wT = np.ascontiguousarray(w.T)

    in_maps = []
    for c in range(N_CORES):
        r0 = c * RB
        f0 = c * FBR
        in_maps.append({
            "am_rows": np.ascontiguousarray(am[r0:r0 + RB, :]),
            "alpha_blk": np.ascontiguousarray(alpha[r0:r0 + RB]),
            "x_full": x,
            "x0_full": x0,
            "x_rows": np.ascontiguousarray(x[r0:r0 + RB, :]),
            "x0_rows": np.ascontiguousarray(x0[r0:r0 + RB, :]),
            "wT_full": wT,
            "wTc": np.ascontiguousarray(wT[:, f0:f0 + FBR]),
            "d_full": d,
        })
    return in_maps


def kernel(**inputs) -> np.ndarray:
    nc = _get_nc()
    in_maps = make_in_maps(inputs)
    res = run_bass_kernel_spmd(nc, in_maps, core_ids=list(range(N_CORES)))
    z = np.concatenate([res.results[c]["z_loc"] for c in range(N_CORES)], axis=0)
    return np.ascontiguousarray(z.astype(np.float32))


if __name__ == "__main__":
    rng = np.random.default_rng(0)
    ins = {
        "x": rng.standard_normal((N, D)).astype(np.float32),
        "x0": rng.standard_normal((N, D)).astype(np.float32),
        "adj": (rng.random((N, N)) / N).astype(np.float32),
        "alpha_train": rng.standard_normal((N,)).astype(np.float32),
        "w": (np.eye(D) + 0.02 * rng.standard_normal((D, D))).astype(np.float32),
        "d": rng.random((D,)).astype(np.float32),
    }
    out = kernel(**ins)
    print("kernel output:", out.shape, out.dtype, float(np.linalg.norm(out)))
